# revision 1
# baseline (speedup 1.0000x reference)
"""Trainium2 Bass kernel for nn_MoEFusion (multi-modal MoE fusion MLP).

Data-parallel across 8 NeuronCores: batch dim (32768) sharded into 8
slices of 4096, all weights (<1 MB) replicated. No collectives.

The v1 all-bf16 pipeline structure with a surgical fp8 x-path:
  - features DMA'd as fp8e4m3 (halves HBM traffic vs bf16)
  - proj: 9 DoubleRow fp8 passes per stripe (vs 18 bf16)
  - x evicted as fp8 (weights pre-scaled x32 against e4m3 denormals;
    the ACT eviction scale divides it back out)
  - gate: 1 DoubleRow + 1 plain fp8 pass (vs 3 bf16)
  - W1: DoubleRow(k0,k1) + plain(k2) fp8 per expert = 16 passes (vs 24)
  - everything downstream of h (gating softmax path, broadcast, gating
    muls, W2/b2, pre, head) stays bf16 exactly as v1: DVE/GPSIMD
    elementwise runs 2x on 2-byte dtypes but half-rate on fp8, and
    bf16 passes interleave with DoubleRow passes at full PE rate.
  - eT bf16 (vs f32r) so the softmax colsum streams 1 col/cycle.
Measured numerics: rel err ~2.9e-3 vs threshold 2e-2.

On-device dataflow per core (feature-major "T" layout):
  featT [3, 768, 4096] fp8 --DMA--> SBUF per 512-col stripe
  xT = concat_m(projT_m + proj_b)  [128x3, 512] fp8
  gateT = exp(gate.T x + b), colsum via ones-matmul, reciprocal,
  gwT = eT * rsum^-1 (bf16); gw rows gathered to partition 0 (DMA),
  broadcast to 128 partitions on GPSIMD; sh_e = h_e * gw_e (DVE)
  fusedT = b2.T gwT + sum_e W2_e.T sh_e  (one PSUM accumulation)
  penT = relu(pre.T fused + pre_b); outT = head.T pen + head_b

Software pipeline: stage-2 (l2 accumulation) of stripe s-3 and
pre/head of older stripes are emitted during stripe s so the PE
stream never waits on the gate-softmax/broadcast chain.
"""

import sys

if "/opt/trn_rl_repo" not in sys.path:
    sys.path.insert(0, "/opt/trn_rl_repo")

from contextlib import ExitStack

import ml_dtypes
import numpy as np

# ---- problem constants (hardcoded per contract) ----
B = 32768
NCORES = 8
BL = B // NCORES  # 4096 per core
STRIPE = 512
NM = 3
NE = 8
D_IN = 768
KIN = D_IN // 128  # 6
D_P = 128
D_X = 384
KX = D_X // 128  # 3

BF16 = ml_dtypes.bfloat16
E4M3 = ml_dtypes.float8_e4m3

WS = 32.0   # fp8 weight pre-scale (e4m3 denormal avoidance)

# ---- fp8 packed weights (columns of [128, W8COLS]) ----
# W1 per expert is [k0|k1|k2|bias] blocks of 128 cols: the bias block
# (row 0 = WS*b1_e) rides in the second half of a DoubleRow pass whose
# moving block is the all-ones chunk of x, so h evictions need no ACT
# bias and can merge across expert pairs.
OFF_PROJ = 0                           # [p, m*768 + k*128 + o] = WS*proj_w
OFF_W1 = OFF_PROJ + NM * KIN * 128     # 2304: [p, e*512 + k*128 + o]
OFF_GATE = OFF_W1 + NE * 4 * 128       # 6400: [p, k*128 + e] (128-padded
W8COLS = OFF_GATE + KX * 128           # 6784   for DoubleRow stride rule)

# ---- bf16 packed weights ----
OFF_PRE = 0                            # [p, 0:64] = pre_w
OFF_HEAD = OFF_PRE + 64                # [p<64, 64:66] = head_w
OFF_ONES = OFF_HEAD + 2                # [p<8, 66:74] = 1.0 (colsum)
OFF_W2B = OFF_ONES + NE                # [p, 74 + e*128 + o] = w2[e, p, o]
OFF_B2B = OFF_W2B + NE * 128           # [p<8, o] = exp_b2[p, o]
WBFCOLS = OFF_B2B + 128                # 1226

# ---- f32 biases (columns of [128, WBCOLS]) ----
OFF_PROJB = 0
OFF_B1 = OFF_PROJB + NM
OFF_GATEB = OFF_B1 + NE
OFF_PREB = OFF_GATEB + 1
OFF_HEADB = OFF_PREB + 1
WBCOLS = OFF_HEADB + 1                 # 14


def pack_weights(inp):
    w8 = np.zeros((128, W8COLS), np.float32)
    pw = np.asarray(inp["proj_w"], np.float32) * WS
    w8[:, OFF_PROJ:OFF_W1] = (
        pw.reshape(NM, KIN, 128, 128).transpose(2, 0, 1, 3).reshape(128, -1)
    )
    w1 = np.asarray(inp["exp_w1"], np.float32) * WS
    blk = np.zeros((128, NE, 4, 128), np.float32)
    blk[:, :, :KX, :] = w1.reshape(NE, KX, 128, 128).transpose(2, 0, 1, 3)
    blk[0, :, KX, :] = np.asarray(inp["exp_b1"], np.float32) * WS
    w8[:, OFF_W1:OFF_GATE] = blk.reshape(128, -1)
    gw = np.asarray(inp["gate_w"], np.float32) * WS
    gblk = np.zeros((128, KX, 128), np.float32)
    gblk[:, :, :NE] = gw.reshape(KX, 128, NE).transpose(1, 0, 2)
    w8[:, OFF_GATE:W8COLS] = gblk.reshape(128, -1)
    w8 = w8.astype(E4M3)

    wbf = np.zeros((128, WBFCOLS), np.float32)
    wbf[:, OFF_PRE:OFF_HEAD] = np.asarray(inp["pre_w"], np.float32)
    wbf[:64, OFF_HEAD:OFF_ONES] = np.asarray(inp["head_w"], np.float32)
    wbf[:NE, OFF_ONES:OFF_W2B] = 1.0
    w2 = np.asarray(inp["exp_w2"], np.float32)
    wbf[:, OFF_W2B:OFF_B2B] = w2.transpose(1, 0, 2).reshape(128, -1)
    wbf[:NE, OFF_B2B:WBFCOLS] = np.asarray(inp["exp_b2"], np.float32)
    wbf = wbf.astype(BF16)

    wbias = np.zeros((128, WBCOLS), np.float32)
    wbias[:, OFF_PROJB:OFF_B1] = np.asarray(inp["proj_b"], np.float32).T
    wbias[:, OFF_B1:OFF_GATEB] = np.asarray(inp["exp_b1"], np.float32).T
    wbias[:NE, OFF_GATEB] = np.asarray(inp["gate_b"], np.float32)
    wbias[:64, OFF_PREB] = np.asarray(inp["pre_b"], np.float32)
    wbias[:2, OFF_HEADB] = np.asarray(inp["head_b"], np.float32)
    return w8, wbf, wbias


def build_program(n_stripes=BL // STRIPE):
    """Build the per-core Bass program (identical on all cores)."""
    import concourse.bacc as bacc
    import concourse.mybir as mybir
    import concourse.tile as tile

    f32 = mybir.dt.float32
    bf16 = mybir.dt.bfloat16
    fp8 = mybir.dt.float8e4
    AF = mybir.ActivationFunctionType
    DR = mybir.MatmulPerfMode.DoubleRow
    ALU = mybir.AluOpType
    bl = n_stripes * STRIPE

    nc = bacc.Bacc(
        "TRN2",
        target_bir_lowering=False,
        debug=False,
        enable_asserts=False,
    )

    featT = nc.dram_tensor("featT", [NM, D_IN, bl], fp8, kind="ExternalInput").ap()
    wmat8 = nc.dram_tensor("wmat8", [128, W8COLS], fp8, kind="ExternalInput").ap()
    wmatbf = nc.dram_tensor("wmatbf", [128, WBFCOLS], bf16, kind="ExternalInput").ap()
    wbias = nc.dram_tensor("wbias", [128, WBCOLS], f32, kind="ExternalInput").ap()
    wones = nc.dram_tensor("wones", [128, STRIPE], fp8, kind="ExternalInput").ap()
    outT = nc.dram_tensor("outT", [2, bl], f32, kind="ExternalOutput").ap()

    with tile.TileContext(nc) as tc, ExitStack() as ctx:
        wp_pool = ctx.enter_context(tc.tile_pool(name="wp", bufs=1))
        feat_pool = ctx.enter_context(tc.tile_pool(name="feat", bufs=12))
        x_pool = ctx.enter_context(tc.tile_pool(name="x", bufs=6))
        gw_pool = ctx.enter_context(tc.tile_pool(name="gw", bufs=4))
        grow_pool = ctx.enter_context(tc.tile_pool(name="grow", bufs=4))
        h_pool = ctx.enter_context(tc.tile_pool(name="h", bufs=10))
        sh_pool = ctx.enter_context(tc.tile_pool(name="sh", bufs=26))
        f_pool = ctx.enter_context(tc.tile_pool(name="f", bufs=2))
        pen_pool = ctx.enter_context(tc.tile_pool(name="pen", bufs=4))
        o_pool = ctx.enter_context(tc.tile_pool(name="o", bufs=4))
        # gb placed last: separates the GPSIMD broadcast-write region from
        # the h/sh regions the DVE muls read, reducing SBUF contention
        gb_pool = ctx.enter_context(tc.tile_pool(name="gb", bufs=6))

        px_pool = ctx.enter_context(tc.tile_pool(name="px", bufs=2, space="PSUM"))
        ph_pool = ctx.enter_context(tc.tile_pool(name="ph", bufs=2, space="PSUM"))
        pf_pool = ctx.enter_context(tc.tile_pool(name="pf", bufs=1, space="PSUM"))
        ps_pool = ctx.enter_context(tc.tile_pool(name="ps", bufs=1, space="PSUM"))

        # preload packed weights once. The two small tensors go first on
        # the sync ring to absorb the queue's cold first-transfer penalty
        # before the feature streams start; proj weights lead the scalar
        # ring so matmuls can start early.
        Bz = wp_pool.tile([128, WBCOLS], f32)
        nc.sync.dma_start(Bz[:], wbias[:])
        Wbf = wp_pool.tile([128, WBFCOLS], bf16)
        nc.sync.dma_start(Wbf[:], wmatbf[:])
        W8 = wp_pool.tile([128, W8COLS], fp8)
        nc.scalar.dma_start(W8[:, :OFF_W1], wmat8[:, :OFF_W1])
        nc.scalar.dma_start(W8[:, OFF_W1:], wmat8[:, OFF_W1:])

        def w8pair(off, m=128, parts=128):
            # stationary [K=128, 2, m] DoubleRow pair at col offset `off`
            return W8[:parts, off:off + 2 * m].rearrange(
                "p (two m) -> p two m", two=2
            )

        def w8s(off, n, parts=128):
            return W8[:parts, off:off + n]

        def wb(off, n, parts=128):
            return Wbf[:parts, off:off + n]

        def bslice(off, parts=128):
            return Bz[:parts, off:off + 1]

        featT_t = featT.rearrange("m (k p) b -> m p k b", p=128)

        pends = []  # (sh, gwT, bsl) of the previous three stripes
        head_pend = None  # (pen, bsl) awaiting its head matmul

        def emit_l2(pend):
            sh, gwT, bsl = pend
            pf = pf_pool.tile([128, STRIPE], f32, tag="pf")
            nc.tensor.matmul(
                pf[:], wb(OFF_B2B, 128, parts=NE), gwT[:],
                start=True, stop=False,
            )
            for e in range(NE):
                nc.tensor.matmul(
                    pf[:],
                    wb(OFF_W2B + e * 128, 128),
                    sh[e][:],
                    start=False,
                    stop=(e == NE - 1),
                )
            fT = f_pool.tile([128, STRIPE], bf16, tag="f")
            nc.scalar.copy(fT[:], pf[:])
            return fT

        def emit_pre(fT):
            pp = ps_pool.tile([64, STRIPE], f32, tag="ps")
            nc.tensor.matmul(pp[:], wb(OFF_PRE, 64), fT[:],
                             start=True, stop=True)
            pen = pen_pool.tile([64, STRIPE], bf16, tag="pen")
            nc.vector.tensor_scalar(
                pen[:], pp[:], bslice(OFF_PREB, parts=64), 0.0,
                op0=ALU.add, op1=ALU.max,
            )
            return pen

        def emit_head2(pen, bsl):
            po = ps_pool.tile([2, STRIPE], f32, tag="ps")
            nc.tensor.matmul(po[:], wb(OFF_HEAD, 2, parts=64), pen[:],
                             start=True, stop=True)
            ot = o_pool.tile([2, STRIPE], f32, tag="o")
            nc.scalar.activation(
                ot[:], po[:], AF.Identity, bias=bslice(OFF_HEADB, parts=2),
                scale=1.0,
            )
            nc.scalar.dma_start(outT[:, bsl], ot[:])

        for s in range(n_stripes):
            bsl = slice(s * STRIPE, (s + 1) * STRIPE)

            # ---- load features (fp8, 0.39 MB per modality) ----
            ft = []
            for m in range(NM):
                t = feat_pool.tile([128, KIN, STRIPE], fp8, tag="feat")
                nc.sync.dma_start(t[:], featT_t[m, :, :, bsl])
                ft.append(t)

            # ---- per-modality projection -> xT chunks (fp8); 4th chunk
            # is all-ones (bias rider for the W1 DoubleRow passes) ----
            xt = x_pool.tile([128, KX + 1, STRIPE], fp8, tag="x")
            nc.scalar.dma_start(xt[:, KX, :], wones[:])
            for m in range(NM):
                px = px_pool.tile([128, STRIPE], f32, tag="px")
                for k in range(KIN // 2):
                    nc.tensor.matmul(
                        px[:],
                        w8pair(OFF_PROJ + m * KIN * 128 + k * 256),
                        ft[m][:, 2 * k:2 * k + 2, :],
                        start=(k == 0),
                        stop=(k == KIN // 2 - 1),
                        perf_mode=DR,
                    )
                nc.scalar.activation(
                    xt[:, m, :], px[:], AF.Identity,
                    bias=bslice(OFF_PROJB + m), scale=1.0 / WS,
                )

            # ---- finish head of an older stripe (pen ACT long done) ----
            if head_pend is not None:
                emit_head2(*head_pend)
                head_pend = None

            # ---- stage-2, three stripes back: l2 accumulation ----
            fT_prev = None
            if len(pends) == 2:
                p0 = pends.pop(0)
                fT_prev = emit_l2(p0)
                pend_bsl = p0[2]

            # ---- gate: softmax over 8 experts ----
            pg = ps_pool.tile([NE, STRIPE], f32, tag="ps")
            nc.tensor.matmul(
                pg[:], w8pair(OFF_GATE)[:, :, :NE], xt[:, 0:2, :],
                start=True, stop=False, perf_mode=DR,
            )
            nc.tensor.matmul(
                pg[:], w8s(OFF_GATE + 256, NE), xt[:, 2, :],
                start=False, stop=True,
            )
            eT = gw_pool.tile([NE, STRIPE], bf16, tag="eT")
            nc.scalar.activation(
                eT[:], pg[:], AF.Exp, bias=bslice(OFF_GATEB, parts=NE),
                scale=1.0 / WS,
            )
            psum_s = ps_pool.tile([NE, STRIPE], f32, tag="ps")
            nc.tensor.matmul(
                psum_s[:], wb(OFF_ONES, NE, parts=NE), eT[:],
                start=True, stop=True,
            )
            rT = gw_pool.tile([NE, STRIPE], f32, tag="rT")
            nc.vector.reciprocal_approx_fast(rT[:], psum_s[:])
            gwT = gw_pool.tile([NE, STRIPE], bf16, tag="gwT")
            nc.vector.tensor_mul(gwT[:], eT[:], rT[:])

            # gather gate rows onto partition 0; broadcast on idle GPSIMD
            grow = grow_pool.tile([1, NE, STRIPE], bf16, tag="grow")
            nc.scalar.dma_start(grow[:], gwT[:])

            # ---- experts: ph = W1.T x (+b1 via ones-rider DoubleRow);
            # h evictions merged across expert pairs (2-bank PSUM ACT);
            # sh = h * gw[e] on DVE from the GPSIMD broadcast ----
            sh = []
            for j in range(NE // 2):
                php = ph_pool.tile([128, 2, STRIPE], f32, tag="ph")
                for i in range(2):
                    e = 2 * j + i
                    off = OFF_W1 + e * 512
                    nc.tensor.matmul(
                        php[:, i, :], w8pair(off), xt[:, 0:2, :],
                        start=True, stop=False, perf_mode=DR,
                    )
                    nc.tensor.matmul(
                        php[:, i, :], w8pair(off + 256), xt[:, 2:4, :],
                        start=False, stop=True, perf_mode=DR,
                    )
                hp = h_pool.tile([128, 2, STRIPE], bf16, tag="h")
                nc.scalar.activation(hp[:], php[:], AF.Relu, scale=1.0 / WS)
                for i in range(2):
                    e = 2 * j + i
                    gb = gb_pool.tile([128, STRIPE], bf16, tag="gb")
                    nc.gpsimd.partition_broadcast(
                        gb[:], grow[0:1, e, :], channels=128
                    )
                    sht = sh_pool.tile([128, STRIPE], bf16, tag="sh")
                    nc.vector.tensor_mul(sht[:], hp[:, i, :], gb[:])
                    sh.append(sht)

            if fT_prev is not None:
                head_pend = (emit_pre(fT_prev), pend_bsl)
            pends.append((sh, gwT, bsl))

        if head_pend is not None:
            emit_head2(*head_pend)
        flush = [(emit_l2(p0), p0[2]) for p0 in pends]
        pens = [(emit_pre(fT), bsl_) for fT, bsl_ in flush]
        for pen, bsl_ in pens:
            emit_head2(pen, bsl_)

    nc.compile()
    return nc


_PROGRAM = None


def _get_program():
    global _PROGRAM
    if _PROGRAM is None:
        _PROGRAM = build_program()
    return _PROGRAM


def make_in_maps(inputs):
    """Host-side shard + layout prep: list of 8 per-core input maps."""
    w8, wbf, wbias = pack_weights(inputs)
    feats = [
        np.asarray(inputs["feat_text"], np.float32),
        np.asarray(inputs["feat_audio"], np.float32),
        np.asarray(inputs["feat_video"], np.float32),
    ]
    in_maps = []
    for c in range(NCORES):
        sl = slice(c * BL, (c + 1) * BL)
        featT = np.stack([np.ascontiguousarray(f[sl].T) for f in feats])
        in_maps.append({
            "featT": featT.astype(E4M3),
            "wmat8": w8,
            "wmatbf": wbf,
            "wbias": wbias,
            "wones": np.ones((128, STRIPE), np.float32).astype(E4M3),
        })
    return in_maps


def run_on_hw(inputs, trace=False):
    from concourse.bass_utils import run_bass_kernel_spmd

    nc = _get_program()
    in_maps = make_in_maps(inputs)
    res = run_bass_kernel_spmd(
        nc, in_maps, core_ids=list(range(NCORES)), trace=trace
    )
    out = np.concatenate([r["outT"].T for r in res.results], axis=0)
    return out, res


def kernel(**inputs):
    out, _ = run_on_hw(inputs, trace=False)
    return out



# revision 3
# speedup vs baseline: 1.2786x; 1.2786x over previous
"""Trainium2 Bass kernel for nn_MoEFusion (multi-modal MoE fusion MLP).

Data-parallel across 8 NeuronCores: batch dim (32768) sharded into 8
slices of 4096, all weights (<1 MB) replicated. No collectives.

v2: expert-interleaved layout kills the GPSIMD broadcast + gather DMA
of v1 (which thrashed the SBUF port GPSIMD shares with the DVE and made
every sh multiply ~2.5x slower than spec):
  - gate stationary weights are column-replicated so partition p holds
    the logit of expert p%8: the gate matmul itself produces the
    gating activations pre-broadcast on all 128 partitions.
  - W1/W2 expert weights are column/row-permuted host-side so W1 pass
    c computes, on partition p, hidden unit c*16+p//8 of expert p%8.
    The per-expert gating multiply then becomes a single plain
    [128,512] DVE tensor_mul sh_c = h_c * eT (2x_1P mode, no
    broadcast), with eT the UNNORMALIZED exp(logits).
  - the softmax denominator is row-summed and broadcast in one ones
    matmul (stationary O8[e,p]=1 ~ p%8==e is implicit: all-ones over
    K=8 rows of eT), and the normalization 1/sum folds into the fT
    eviction multiply after the W2 accumulation (W2 is linear in e).
  - b1 applied via ACT bias during per-chunk h eviction (no ones-rider
    column, no wones DMA); b2 via one extra accumulation pass with
    moving eT[0:8].
  - features DMA'd as fp8 in per-(modality, stripe) host-contiguous
    chunks: 128 descriptors x 3KB instead of 768 x 512B, which removes
    the multi-microsecond DIRECT2D descriptor storms of v1's ramp.
Measured numerics: rel err ~3e-3 vs threshold 2e-2.

Per-stripe dataflow (feature-major "T" layout, 512-token stripes):
  xT = concat_m(projT_m + proj_b)      [128*3, 512] fp8   (9 DR MM)
  pg = gateRep.T x                     [128, 512]         (2 MM)
  eT = exp(pg/WS + gb[p%8])            bf16 ACT
  rs = ones8.T eT[0:8]                 [128, 512]         (1 MM)
  rb = 1/rs                            f32 DVE
  ph_c = W1p_c.T x; h_c = relu(ph_c/WS + b1_c)  (16 MM + 8 ACT)
  sh_c = h_c * eT                      bf16 DVE 2x
  pf = b2.T eT[0:8] + sum_c W2p_c.T sh_c        (9 MM, stripe s-2)
  fT = pf * rb                         bf16 DVE
  penT = relu(pre.T fT + pre_b); outT = head.T pen + head_b
"""

import sys

if "/opt/trn_rl_repo" not in sys.path:
    sys.path.insert(0, "/opt/trn_rl_repo")

from contextlib import ExitStack

import ml_dtypes
import numpy as np

# ---- problem constants (hardcoded per contract) ----
B = 32768
NCORES = 8
BL = B // NCORES  # 4096 per core
STRIPE = 512
NSTRIPES = BL // STRIPE  # 8
NM = 3
NE = 8
D_IN = 768
KIN = D_IN // 128  # 6
D_P = 128
D_X = 384
KX = D_X // 128  # 3
NC_H = 8  # W1/W2 hidden chunks (8 chunks x 16 hidden/expert)

BF16 = ml_dtypes.bfloat16
E4M3 = ml_dtypes.float8_e4m3

WS = 32.0   # fp8 weight pre-scale (e4m3 denormal avoidance)

# ---- fp8 packed weights (columns of [128, W8COLS]) ----
OFF_PROJ = 0                           # [p, m*768 + k*128 + o] = WS*proj_w
OFF_W1 = OFF_PROJ + NM * KIN * 128     # 2304: [p, c*384 + kb*128 + o]
OFF_GATE = OFF_W1 + NC_H * 3 * 128     # 5376: [p, kb*128 + o], col o = gate_w[., o%8]
W8COLS = OFF_GATE + KX * 128           # 5760

# ---- bf16 packed weights ----
OFF_PRE = 0                            # [p, 0:64] = pre_w
OFF_HEAD = OFF_PRE + 64                # [p<64, 64:66] = head_w
OFF_ONES = OFF_HEAD + 2                # [p<8, 66:194] = 1.0 (row-sum bcast)
OFF_W2B = OFF_ONES + 128               # [p, 194 + c*128 + o] = w2 permuted
OFF_B2B = OFF_W2B + NC_H * 128         # [p<8, o] = exp_b2[p, o]
WBFCOLS = OFF_B2B + 128                # 1346

# ---- f32 biases (columns of [128, WBCOLS]) ----
OFF_PROJB = 0
OFF_GATEB = OFF_PROJB + NM             # gate_b[p%8], 128 rows
OFF_B1 = OFF_GATEB + 1                 # col c, row p = exp_b1[p%8, c*16+p//8]
OFF_PREB = OFF_B1 + NC_H
OFF_HEADB = OFF_PREB + 1
WBCOLS = OFF_HEADB + 1                 # 14


def pack_weights(inp):
    w8 = np.zeros((128, W8COLS), np.float32)
    pw = np.asarray(inp["proj_w"], np.float32) * WS
    w8[:, OFF_PROJ:OFF_W1] = (
        pw.reshape(NM, KIN, 128, 128).transpose(2, 0, 1, 3).reshape(128, -1)
    )
    # W1 permuted: pass c, k-block kb, stationary col o = (e=o%8, h=c*16+o//8)
    w1 = np.asarray(inp["exp_w1"], np.float32) * WS  # [NE, D_X, D_P]
    p = np.arange(128)
    e_of = p % NE
    hsub = p // NE  # 0..15
    blk = np.zeros((128, NC_H, KX, 128), np.float32)
    for c in range(NC_H):
        # [D_X, 128] -> [KX, 128(kp), 128(o)]
        wc = w1[e_of, :, c * 16 + hsub].T  # [D_X, 128]
        blk[:, c, :, :] = wc.reshape(KX, 128, 128).transpose(1, 0, 2)
    w8[:, OFF_W1:OFF_GATE] = blk.reshape(128, -1)
    # gate replicated: col o = gate_w[., o%8]
    gw = np.asarray(inp["gate_w"], np.float32) * WS  # [D_X, NE]
    grep = gw[:, e_of]  # [D_X, 128]
    w8[:, OFF_GATE:W8COLS] = grep.reshape(KX, 128, 128).transpose(1, 0, 2).reshape(
        128, -1
    )
    w8 = w8.astype(E4M3)

    wbf = np.zeros((128, WBFCOLS), np.float32)
    wbf[:, OFF_PRE:OFF_HEAD] = np.asarray(inp["pre_w"], np.float32)
    wbf[:64, OFF_HEAD:OFF_ONES] = np.asarray(inp["head_w"], np.float32)
    wbf[:NE, OFF_ONES:OFF_W2B] = 1.0
    w2 = np.asarray(inp["exp_w2"], np.float32)  # [NE, D_P, D_P]
    for c in range(NC_H):
        # row p' = w2[p'%8, c*16+p'//8, :]
        wbf[:, OFF_W2B + c * 128:OFF_W2B + (c + 1) * 128] = w2[
            e_of, c * 16 + hsub, :
        ]
    wbf[:NE, OFF_B2B:WBFCOLS] = np.asarray(inp["exp_b2"], np.float32)
    wbf = wbf.astype(BF16)

    wbias = np.zeros((128, WBCOLS), np.float32)
    wbias[:, OFF_PROJB:OFF_GATEB] = np.asarray(inp["proj_b"], np.float32).T
    gb = np.asarray(inp["gate_b"], np.float32)
    wbias[:, OFF_GATEB] = gb[e_of]
    b1 = np.asarray(inp["exp_b1"], np.float32)  # [NE, D_P]
    for c in range(NC_H):
        wbias[:, OFF_B1 + c] = b1[e_of, c * 16 + hsub]
    wbias[:64, OFF_PREB] = np.asarray(inp["pre_b"], np.float32)
    wbias[:2, OFF_HEADB] = np.asarray(inp["head_b"], np.float32)
    return w8, wbf, wbias


def build_program(n_stripes=NSTRIPES):
    """Build the per-core Bass program (identical on all cores)."""
    import concourse.bacc as bacc
    import concourse.mybir as mybir
    import concourse.tile as tile

    f32 = mybir.dt.float32
    bf16 = mybir.dt.bfloat16
    fp8 = mybir.dt.float8e4
    AF = mybir.ActivationFunctionType
    DR = mybir.MatmulPerfMode.DoubleRow
    ALU = mybir.AluOpType
    bl = n_stripes * STRIPE

    nc = bacc.Bacc(
        "TRN2",
        target_bir_lowering=False,
        debug=False,
        enable_asserts=False,
    )

    # features packed host-side as [m][s][p][k][b]: per-partition rows of
    # 6*512 fp8 bytes are contiguous -> 128 descriptors per DMA.
    featS = nc.dram_tensor(
        "featS", [NM, n_stripes, 128, KIN * STRIPE], fp8, kind="ExternalInput"
    ).ap()
    wmat8 = nc.dram_tensor("wmat8", [128, W8COLS], fp8, kind="ExternalInput").ap()
    wmatbf = nc.dram_tensor("wmatbf", [128, WBFCOLS], bf16, kind="ExternalInput").ap()
    wbias = nc.dram_tensor("wbias", [128, WBCOLS], f32, kind="ExternalInput").ap()
    outT = nc.dram_tensor("outT", [2, bl], f32, kind="ExternalOutput").ap()

    with tile.TileContext(nc) as tc, ExitStack() as ctx:
        wp_pool = ctx.enter_context(tc.tile_pool(name="wp", bufs=1))
        feat_pool = ctx.enter_context(tc.tile_pool(name="feat", bufs=9))
        x_pool = ctx.enter_context(tc.tile_pool(name="x", bufs=3))
        e_pool = ctx.enter_context(tc.tile_pool(name="e", bufs=3))
        r_pool = ctx.enter_context(tc.tile_pool(name="r", bufs=3))
        h_pool = ctx.enter_context(tc.tile_pool(name="h", bufs=4))
        sh_pool = ctx.enter_context(tc.tile_pool(name="sh", bufs=24))
        f_pool = ctx.enter_context(tc.tile_pool(name="f", bufs=2))
        pen_pool = ctx.enter_context(tc.tile_pool(name="pen", bufs=2))
        o_pool = ctx.enter_context(tc.tile_pool(name="o", bufs=2))

        px_pool = ctx.enter_context(tc.tile_pool(name="px", bufs=2, space="PSUM"))
        ph_pool = ctx.enter_context(tc.tile_pool(name="ph", bufs=3, space="PSUM"))
        pf_pool = ctx.enter_context(tc.tile_pool(name="pf", bufs=1, space="PSUM"))
        ps_pool = ctx.enter_context(tc.tile_pool(name="ps", bufs=2, space="PSUM"))

        # preload packed weights once; proj block leads so matmuls start
        # early. Features stream on the sync ring.
        W8 = wp_pool.tile([128, W8COLS], fp8)
        nc.scalar.dma_start(W8[:, :OFF_W1], wmat8[:, :OFF_W1])
        nc.scalar.dma_start(W8[:, OFF_W1:], wmat8[:, OFF_W1:])
        Wbf = wp_pool.tile([128, WBFCOLS], bf16)
        nc.scalar.dma_start(Wbf[:], wmatbf[:])
        Bz = wp_pool.tile([128, WBCOLS], f32)
        nc.scalar.dma_start(Bz[:], wbias[:])

        def w8pair(off, m=128, parts=128):
            # stationary [K=128, 2, m] DoubleRow pair at col offset `off`
            return W8[:parts, off:off + 2 * m].rearrange(
                "p (two m) -> p two m", two=2
            )

        def w8s(off, n, parts=128):
            return W8[:parts, off:off + n]

        def wb(off, n, parts=128):
            return Wbf[:parts, off:off + n]

        def bslice(off, parts=128):
            return Bz[:parts, off:off + 1]

        pends = []  # (sh_list, eT, rb, bsl) of stripes awaiting stage-2
        head_pend = None  # (pen, bsl) awaiting its head matmul

        def emit_l2(pend):
            sh, eT, rb, bsl = pend
            pf = pf_pool.tile([128, STRIPE], f32, tag="pf")
            nc.tensor.matmul(
                pf[:], wb(OFF_B2B, 128, parts=NE), eT[:NE, :],
                start=True, stop=False,
            )
            for c in range(NC_H):
                nc.tensor.matmul(
                    pf[:],
                    wb(OFF_W2B + c * 128, 128),
                    sh[c][:],
                    start=False,
                    stop=(c == NC_H - 1),
                )
            fT = f_pool.tile([128, STRIPE], bf16, tag="f")
            nc.vector.tensor_mul(fT[:], pf[:], rb[:])
            return fT

        def emit_pre(fT):
            pp = px_pool.tile([64, STRIPE], f32, tag="px")
            nc.tensor.matmul(pp[:], wb(OFF_PRE, 64), fT[:],
                             start=True, stop=True)
            pen = pen_pool.tile([64, STRIPE], bf16, tag="pen")
            nc.vector.tensor_scalar(
                pen[:], pp[:], bslice(OFF_PREB, parts=64), 0.0,
                op0=ALU.add, op1=ALU.max,
            )
            return pen

        def emit_head2(pen, bsl):
            po = px_pool.tile([2, STRIPE], f32, tag="px")
            nc.tensor.matmul(po[:], wb(OFF_HEAD, 2, parts=64), pen[:],
                             start=True, stop=True)
            ot = o_pool.tile([2, STRIPE], f32, tag="o")
            nc.scalar.activation(
                ot[:], po[:], AF.Identity, bias=bslice(OFF_HEADB, parts=2),
                scale=1.0,
            )
            nc.sync.dma_start(outT[:, bsl], ot[:])

        # prefetch features a few stripes ahead
        ftiles = {}

        def fetch(s):
            if s >= n_stripes:
                return
            for m in range(NM):
                t = feat_pool.tile([128, KIN, STRIPE], fp8, tag="feat")
                nc.sync.dma_start(
                    t[:].rearrange("p k b -> p (k b)"), featS[m, s, :, :]
                )
                ftiles[(m, s)] = t

        for s in range(3):
            fetch(s)

        for s in range(n_stripes):
            bsl = slice(s * STRIPE, (s + 1) * STRIPE)
            fetch(s + 3)

            # ---- per-modality projection -> xT chunks (fp8) ----
            xt = x_pool.tile([128, KX, STRIPE], fp8, tag="x")
            for m in range(NM):
                ft = ftiles.pop((m, s))
                px = px_pool.tile([128, STRIPE], f32, tag="px")
                for k in range(KIN // 2):
                    nc.tensor.matmul(
                        px[:],
                        w8pair(OFF_PROJ + m * KIN * 128 + k * 256),
                        ft[:, 2 * k:2 * k + 2, :],
                        start=(k == 0),
                        stop=(k == KIN // 2 - 1),
                        perf_mode=DR,
                    )
                nc.scalar.activation(
                    xt[:, m, :], px[:], AF.Identity,
                    bias=bslice(OFF_PROJB + m), scale=1.0 / WS,
                )

            # ---- gate logits on all 128 partitions (expert p%8) ----
            pg = ps_pool.tile([128, STRIPE], f32, tag="ps")
            nc.tensor.matmul(
                pg[:], w8pair(OFF_GATE), xt[:, 0:2, :],
                start=True, stop=False, perf_mode=DR,
            )
            nc.tensor.matmul(
                pg[:], w8s(OFF_GATE + 256, 128), xt[:, 2, :],
                start=False, stop=True,
            )
            eT = e_pool.tile([128, STRIPE], bf16, tag="eT")
            nc.scalar.activation(
                eT[:], pg[:], AF.Exp, bias=bslice(OFF_GATEB),
                scale=1.0 / WS,
            )
            # row-sum broadcast: all-ones over the K=8 leading rows of eT
            prs = ps_pool.tile([128, STRIPE], f32, tag="ps")
            nc.tensor.matmul(
                prs[:], wb(OFF_ONES, 128, parts=NE), eT[:NE, :],
                start=True, stop=True,
            )
            rb = r_pool.tile([128, STRIPE], f32, tag="rb")
            nc.vector.reciprocal_approx_fast(rb[:], prs[:])

            # ---- finish head of an older stripe ----
            if head_pend is not None:
                emit_head2(*head_pend)
                head_pend = None

            # ---- stage-2, two stripes back: W2 accumulation ----
            fT_prev = None
            if len(pends) == 2:
                p0 = pends.pop(0)
                fT_prev = emit_l2(p0)
                pend_bsl = p0[3]

            # ---- experts: ph_c = W1p_c.T x; h = relu; sh = h * eT ----
            sh = []
            for c in range(NC_H):
                php = ph_pool.tile([128, STRIPE], f32, tag="ph")
                off = OFF_W1 + c * 384
                nc.tensor.matmul(
                    php[:], w8pair(off), xt[:, 0:2, :],
                    start=True, stop=False, perf_mode=DR,
                )
                nc.tensor.matmul(
                    php[:], w8s(off + 256, 128), xt[:, 2, :],
                    start=False, stop=True,
                )
                hc = h_pool.tile([128, STRIPE], bf16, tag="h")
                nc.scalar.activation(
                    hc[:], php[:], AF.Relu, bias=bslice(OFF_B1 + c),
                    scale=1.0 / WS,
                )
                sht = sh_pool.tile([128, STRIPE], bf16, tag="sh")
                nc.vector.tensor_mul(sht[:], hc[:], eT[:])
                sh.append(sht)

            if fT_prev is not None:
                head_pend = (emit_pre(fT_prev), pend_bsl)
            pends.append((sh, eT, rb, bsl))

        if head_pend is not None:
            emit_head2(*head_pend)
            head_pend = None
        flush = [(emit_l2(p0), p0[3]) for p0 in pends]
        pens = [(emit_pre(fT), bsl_) for fT, bsl_ in flush]
        for pen, bsl_ in pens:
            emit_head2(pen, bsl_)

    nc.compile()
    return nc


_PROGRAM = None


def _get_program():
    global _PROGRAM
    if _PROGRAM is None:
        _PROGRAM = build_program()
    return _PROGRAM


def make_in_maps(inputs):
    """Host-side shard + layout prep: list of 8 per-core input maps."""
    w8, wbf, wbias = pack_weights(inputs)
    feats = [
        np.asarray(inputs["feat_text"], np.float32),
        np.asarray(inputs["feat_audio"], np.float32),
        np.asarray(inputs["feat_video"], np.float32),
    ]
    in_maps = []
    for cid in range(NCORES):
        sl = slice(cid * BL, (cid + 1) * BL)
        # [m][s][p][k][b]: featS[m,s,p,k*512+b] = feat_m[s*512+b, k*128+p]
        featT = np.stack([np.ascontiguousarray(f[sl].T) for f in feats])
        # featT: [NM, 768, 4096] -> [NM, KIN(k), 128(p), NSTRIPES(s), 512(b)]
        fs = featT.reshape(NM, KIN, 128, NSTRIPES, STRIPE)
        fs = fs.transpose(0, 3, 2, 1, 4).reshape(NM, NSTRIPES, 128, KIN * STRIPE)
        in_maps.append({
            "featS": np.ascontiguousarray(fs).astype(E4M3),
            "wmat8": w8,
            "wmatbf": wbf,
            "wbias": wbias,
        })
    return in_maps


def run_on_hw(inputs, trace=False):
    from concourse.bass_utils import run_bass_kernel_spmd

    nc = _get_program()
    in_maps = make_in_maps(inputs)
    res = run_bass_kernel_spmd(
        nc, in_maps, core_ids=list(range(NCORES)), trace=trace
    )
    out = np.concatenate([r["outT"].T for r in res.results], axis=0)
    return out, res


def kernel(**inputs):
    out, _ = run_on_hw(inputs, trace=False)
    return out


# revision 4
# speedup vs baseline: 1.3344x; 1.0437x over previous
"""Trainium2 Bass kernel for nn_MoEFusion (multi-modal MoE fusion MLP).

Data-parallel across 8 NeuronCores: batch dim (32768) sharded into 8
slices of 4096, all weights (<1 MB) replicated. No collectives.

v3: expert-interleaved layout (no GPSIMD broadcast, no gather DMA) +
fp8 DoubleRow W2 + paired h evictions + 1-stripe stage-2 lag.

Key layout trick (from v2): gate stationary weights are column-
replicated so partition p holds the logit of expert p%8; W1/W2 expert
weights are column/row-permuted host-side so W1 pass c computes, on
partition p, hidden unit c*16+p//8 of expert p%8. The per-expert
gating multiply is then a plain [128,512] DVE tensor_mul against the
unnormalized exp tile (2x_1P, no broadcast); the softmax denominator
is row-summed+broadcast by one ones-matmul and its reciprocal folds
into the fT eviction multiply after W2 (W2 is linear in e).

Scale bookkeeping (all cancel exactly in fT = pf * rb):
  W1' = 32 w1 (fp8), b1 rider row = 32 b1 (fp8), ACT h evict scale 8
    -> h' = 256 relu(x w1 + b1)
  e' = exp(logits + gate_b - ln 32) = e/32  (ACT bias col)
  sh' = h' * e' = 8 (h*e)  (fp8, above e4m3 denormals)
  W2' = 32 w2 (fp8 DR pairs)  -> W2 terms = 256 X1
  b2' = 8192 b2 (bf16, moving e'[0:8]) -> 256 X2
  ones = 8192 -> prs = 256 sum(e); rb = 1/(256 sum(e))
  fT = pf * rb = (sum_e e*(eo+b2))/sum(e) = fused   (exact)

DMA: features packed host-side per (modality, stripe) with contiguous
3KB per-partition rows (128 descriptors/DMA); the tiny bias tensor
loads FIRST on the scalar ring so the first x eviction is never
blocked behind the weight+feature streams.
"""

import sys

if "/opt/trn_rl_repo" not in sys.path:
    sys.path.insert(0, "/opt/trn_rl_repo")

import math
from contextlib import ExitStack

import ml_dtypes
import numpy as np

# ---- problem constants (hardcoded per contract) ----
B = 32768
NCORES = 8
BL = B // NCORES  # 4096 per core
STRIPE = 512
NSTRIPES = BL // STRIPE  # 8
NM = 3
NE = 8
D_IN = 768
KIN = D_IN // 128  # 6
D_P = 128
D_X = 384
KX = D_X // 128  # 3
NC_H = 8  # W1/W2 hidden chunks (8 chunks x 16 hidden/expert)

BF16 = ml_dtypes.bfloat16
E4M3 = ml_dtypes.float8_e4m3

WS = 32.0    # fp8 weight pre-scale (e4m3 denormal avoidance)
HS = 8.0     # extra h eviction scale (h' = 256 h)
VONES = 8192.0

# ---- fp8 packed weights (columns of [128, W8COLS]) ----
OFF_PROJ = 0                           # [p, m*768 + k*128 + o] = 32*proj_w
OFF_W1 = OFF_PROJ + NM * KIN * 128     # 2304: [p, c*512 + blk*128 + o]
#   per chunk c: [k0 | k1 | k2 | rider], rider row0 = 32*b1 permuted
OFF_GATE = OFF_W1 + NC_H * 4 * 128     # 6400: [p, kb*128+o], col o=gate_w[.,o%8]
OFF_W2F = OFF_GATE + KX * 128          # 6784: 4 DR pairs [c2*256 + blk*128 + o]
W8COLS = OFF_W2F + NC_H * 128          # 7808

# ---- bf16 packed weights ----
OFF_PRE = 0                            # [p, 0:64] = pre_w
OFF_HEAD = OFF_PRE + 64                # [p<64, 64:66] = head_w
OFF_ONES = OFF_HEAD + 2                # [p<8, 66:194] = 8192.0 (row-sum bcast)
OFF_B2B = OFF_ONES + 128               # [p<8, o] = 8192*exp_b2[p, o]
WBFCOLS = OFF_B2B + 128                # 322

# ---- f32 biases (columns of [128, WBCOLS]) ----
OFF_PROJB = 0
OFF_GATEB = OFF_PROJB + NM             # gate_b[p%8] - ln 32, 128 rows
OFF_PREB = OFF_GATEB + 1
OFF_HEADB = OFF_PREB + 1
WBCOLS = OFF_HEADB + 1                 # 6


def pack_weights(inp):
    p = np.arange(128)
    e_of = p % NE
    hsub = p // NE  # 0..15

    w8 = np.zeros((128, W8COLS), np.float32)
    pw = np.asarray(inp["proj_w"], np.float32) * WS
    w8[:, OFF_PROJ:OFF_W1] = (
        pw.reshape(NM, KIN, 128, 128).transpose(2, 0, 1, 3).reshape(128, -1)
    )
    # W1 permuted: pass c, stationary col o = (e=o%8, h=c*16+o//8)
    w1 = np.asarray(inp["exp_w1"], np.float32) * WS  # [NE, D_X, D_P]
    b1 = np.asarray(inp["exp_b1"], np.float32)
    blk = np.zeros((128, NC_H, 4, 128), np.float32)
    for c in range(NC_H):
        wc = w1[e_of, :, c * 16 + hsub].T  # [D_X, 128]
        blk[:, c, :KX, :] = wc.reshape(KX, 128, 128).transpose(1, 0, 2)
        blk[0, c, KX, :] = b1[e_of, c * 16 + hsub] * WS  # ones-rider row
    w8[:, OFF_W1:OFF_GATE] = blk.reshape(128, -1)
    # gate replicated: col o = gate_w[., o%8]
    gw = np.asarray(inp["gate_w"], np.float32) * WS  # [D_X, NE]
    grep = gw[:, e_of]  # [D_X, 128]
    w8[:, OFF_GATE:OFF_W2F] = grep.reshape(KX, 128, 128).transpose(1, 0, 2).reshape(
        128, -1
    )
    # W2 fp8 DR pairs: chunk pair (2j, 2j+1), row p' = (e=p'%8, h=c*16+p'//8)
    w2 = np.asarray(inp["exp_w2"], np.float32) * WS  # [NE, D_P, D_P]
    w2blk = np.zeros((128, NC_H, 128), np.float32)
    for c in range(NC_H):
        w2blk[:, c, :] = w2[e_of, c * 16 + hsub, :]
    w8[:, OFF_W2F:W8COLS] = w2blk.reshape(128, -1)
    w8 = w8.astype(E4M3)

    wbf = np.zeros((128, WBFCOLS), np.float32)
    wbf[:, OFF_PRE:OFF_HEAD] = np.asarray(inp["pre_w"], np.float32)
    wbf[:64, OFF_HEAD:OFF_ONES] = np.asarray(inp["head_w"], np.float32)
    wbf[:NE, OFF_ONES:OFF_B2B] = VONES
    wbf[:NE, OFF_B2B:WBFCOLS] = np.asarray(inp["exp_b2"], np.float32) * VONES
    wbf = wbf.astype(BF16)

    wbias = np.zeros((128, WBCOLS), np.float32)
    wbias[:, OFF_PROJB:OFF_GATEB] = np.asarray(inp["proj_b"], np.float32).T
    gb = np.asarray(inp["gate_b"], np.float32)
    wbias[:, OFF_GATEB] = gb[e_of] - math.log(WS)
    wbias[:64, OFF_PREB] = np.asarray(inp["pre_b"], np.float32)
    wbias[:2, OFF_HEADB] = np.asarray(inp["head_b"], np.float32)
    return w8, wbf, wbias


def build_program(n_stripes=NSTRIPES):
    """Build the per-core Bass program (identical on all cores)."""
    import concourse.bacc as bacc
    import concourse.mybir as mybir
    import concourse.tile as tile

    f32 = mybir.dt.float32
    bf16 = mybir.dt.bfloat16
    fp8 = mybir.dt.float8e4
    AF = mybir.ActivationFunctionType
    DR = mybir.MatmulPerfMode.DoubleRow
    ALU = mybir.AluOpType
    bl = n_stripes * STRIPE

    nc = bacc.Bacc(
        "TRN2",
        target_bir_lowering=False,
        debug=False,
        enable_asserts=False,
    )

    featS = nc.dram_tensor(
        "featS", [NM, n_stripes, 128, KIN * STRIPE], fp8, kind="ExternalInput"
    ).ap()
    wmat8 = nc.dram_tensor("wmat8", [128, W8COLS], fp8, kind="ExternalInput").ap()
    wmatbf = nc.dram_tensor("wmatbf", [128, WBFCOLS], bf16, kind="ExternalInput").ap()
    wbias = nc.dram_tensor("wbias", [128, WBCOLS], f32, kind="ExternalInput").ap()
    outT = nc.dram_tensor("outT", [2, bl], f32, kind="ExternalOutput").ap()

    with tile.TileContext(nc) as tc, ExitStack() as ctx:
        wp_pool = ctx.enter_context(tc.tile_pool(name="wp", bufs=1))
        feat_pool = ctx.enter_context(tc.tile_pool(name="feat", bufs=9))
        x_pool = ctx.enter_context(tc.tile_pool(name="x", bufs=3))
        e_pool = ctx.enter_context(tc.tile_pool(name="e", bufs=3))
        r_pool = ctx.enter_context(tc.tile_pool(name="r", bufs=3))
        h_pool = ctx.enter_context(tc.tile_pool(name="h", bufs=3))
        sh_pool = ctx.enter_context(tc.tile_pool(name="sh", bufs=10))
        f_pool = ctx.enter_context(tc.tile_pool(name="f", bufs=2))
        pen_pool = ctx.enter_context(tc.tile_pool(name="pen", bufs=2))
        o_pool = ctx.enter_context(tc.tile_pool(name="o", bufs=2))

        px_pool = ctx.enter_context(tc.tile_pool(name="px", bufs=2, space="PSUM"))
        ph_pool = ctx.enter_context(tc.tile_pool(name="ph", bufs=2, space="PSUM"))
        pf_pool = ctx.enter_context(tc.tile_pool(name="pf", bufs=1, space="PSUM"))
        ps_pool = ctx.enter_context(tc.tile_pool(name="ps", bufs=1, space="PSUM"))

        # bias columns first (tiny, unblocks the first x eviction), then
        # proj weights so matmuls start early; the rest follows.
        Bz = wp_pool.tile([128, WBCOLS], f32)
        nc.scalar.dma_start(Bz[:], wbias[:])
        W8 = wp_pool.tile([128, W8COLS], fp8)
        nc.scalar.dma_start(W8[:, :OFF_W1], wmat8[:, :OFF_W1])
        nc.scalar.dma_start(W8[:, OFF_W1:], wmat8[:, OFF_W1:])
        Wbf = wp_pool.tile([128, WBFCOLS], bf16)
        nc.scalar.dma_start(Wbf[:], wmatbf[:])

        def w8pair(off, m=128, parts=128):
            # stationary [K=128, 2, m] DoubleRow pair at col offset `off`
            return W8[:parts, off:off + 2 * m].rearrange(
                "p (two m) -> p two m", two=2
            )

        def w8s(off, n, parts=128):
            return W8[:parts, off:off + n]

        def wb(off, n, parts=128):
            return Wbf[:parts, off:off + n]

        def bslice(off, parts=128):
            return Bz[:parts, off:off + 1]

        pends = []  # (sh_pairs, eT, rb, bsl) awaiting stage-2 (lag 1)
        head_pend = None  # (pen, bsl) awaiting its head matmul

        def emit_l2(pend):
            sh, eT, rb, bsl = pend
            pf = pf_pool.tile([128, STRIPE], f32, tag="pf")
            nc.tensor.matmul(
                pf[:], wb(OFF_B2B, 128, parts=NE), eT[:NE, :],
                start=True, stop=False,
            )
            for j in range(NC_H // 2):
                nc.tensor.matmul(
                    pf[:], w8pair(OFF_W2F + j * 256), sh[j][:],
                    start=False, stop=(j == NC_H // 2 - 1), perf_mode=DR,
                )
            fT = f_pool.tile([128, STRIPE], bf16, tag="f")
            nc.vector.tensor_mul(fT[:], pf[:], rb[:])
            return fT

        def emit_pre(fT):
            pp = px_pool.tile([64, STRIPE], f32, tag="px")
            nc.tensor.matmul(pp[:], wb(OFF_PRE, 64), fT[:],
                             start=True, stop=True)
            pen = pen_pool.tile([64, STRIPE], bf16, tag="pen")
            nc.vector.tensor_scalar(
                pen[:], pp[:], bslice(OFF_PREB, parts=64), 0.0,
                op0=ALU.add, op1=ALU.max,
            )
            return pen

        def emit_head2(pen, bsl):
            po = px_pool.tile([2, STRIPE], f32, tag="px")
            nc.tensor.matmul(po[:], wb(OFF_HEAD, 2, parts=64), pen[:],
                             start=True, stop=True)
            ot = o_pool.tile([2, STRIPE], f32, tag="o")
            nc.scalar.activation(
                ot[:], po[:], AF.Identity, bias=bslice(OFF_HEADB, parts=2),
                scale=1.0,
            )
            nc.sync.dma_start(outT[:, bsl], ot[:])

        ftiles = {}

        def fetch(s):
            if s >= n_stripes:
                return
            for m in range(NM):
                t = feat_pool.tile([128, KIN, STRIPE], fp8, tag="feat")
                nc.sync.dma_start(
                    t[:].rearrange("p k b -> p (k b)"), featS[m, s, :, :]
                )
                ftiles[(m, s)] = t

        for s in range(3):
            fetch(s)

        for s in range(n_stripes):
            bsl = slice(s * STRIPE, (s + 1) * STRIPE)
            fetch(s + 3)

            # ---- per-modality projection -> xT chunks (fp8) ----
            xt = x_pool.tile([128, KX + 1, STRIPE], fp8, tag="x")
            nc.gpsimd.memset(xt[:, KX, :], 1.0)  # ones chunk (b1 rider)
            for m in range(NM):
                ft = ftiles.pop((m, s))
                px = px_pool.tile([128, STRIPE], f32, tag="px")
                for k in range(KIN // 2):
                    nc.tensor.matmul(
                        px[:],
                        w8pair(OFF_PROJ + m * KIN * 128 + k * 256),
                        ft[:, 2 * k:2 * k + 2, :],
                        start=(k == 0),
                        stop=(k == KIN // 2 - 1),
                        perf_mode=DR,
                    )
                nc.scalar.activation(
                    xt[:, m, :], px[:], AF.Identity,
                    bias=bslice(OFF_PROJB + m), scale=1.0 / WS,
                )

            # ---- gate logits on all 128 partitions (expert p%8) ----
            pg = ps_pool.tile([128, STRIPE], f32, tag="ps")
            nc.tensor.matmul(
                pg[:], w8pair(OFF_GATE), xt[:, 0:2, :],
                start=True, stop=False, perf_mode=DR,
            )
            nc.tensor.matmul(
                pg[:], w8s(OFF_GATE + 256, 128), xt[:, 2, :],
                start=False, stop=True,
            )
            eT = e_pool.tile([128, STRIPE], bf16, tag="eT")
            nc.scalar.activation(
                eT[:], pg[:], AF.Exp, bias=bslice(OFF_GATEB),
                scale=1.0 / WS,
            )
            # row-sum broadcast: prs = 8192 * sum_e e'[e] = 256 sum(e)
            prs = ps_pool.tile([128, STRIPE], f32, tag="ps")
            nc.tensor.matmul(
                prs[:], wb(OFF_ONES, 128, parts=NE), eT[:NE, :],
                start=True, stop=True,
            )
            rb = r_pool.tile([128, STRIPE], f32, tag="rb")
            nc.vector.reciprocal_approx_fast(rb[:], prs[:])

            # ---- finish head of stripe s-2 ----
            if head_pend is not None:
                emit_head2(*head_pend)
                head_pend = None

            # ---- stage-2 of stripe s-1: W2 accumulation ----
            fT_prev = None
            if pends:
                p0 = pends.pop(0)
                fT_prev = emit_l2(p0)
                pend_bsl = p0[3]

            # ---- experts: ph = W1.T x (+b1 rider); h' = 256h; sh' = 8he --
            sh = []
            for j in range(NC_H // 2):
                php = ph_pool.tile([128, 2, STRIPE], f32, tag="ph")
                for i in range(2):
                    c = 2 * j + i
                    off = OFF_W1 + c * 512
                    nc.tensor.matmul(
                        php[:, i, :], w8pair(off), xt[:, 0:2, :],
                        start=True, stop=False, perf_mode=DR,
                    )
                    nc.tensor.matmul(
                        php[:, i, :], w8pair(off + 256), xt[:, 2:4, :],
                        start=False, stop=True, perf_mode=DR,
                    )
                hp = h_pool.tile([128, 2, STRIPE], bf16, tag="h")
                nc.scalar.activation(hp[:], php[:], AF.Relu, scale=HS)
                sht = sh_pool.tile([128, 2, STRIPE], fp8, tag="sh")
                for i in range(2):
                    nc.vector.tensor_mul(sht[:, i, :], hp[:, i, :], eT[:])
                sh.append(sht)

            if fT_prev is not None:
                head_pend = (emit_pre(fT_prev), pend_bsl)
            pends.append((sh, eT, rb, bsl))

        if head_pend is not None:
            emit_head2(*head_pend)
            head_pend = None
        flush = [(emit_l2(p0), p0[3]) for p0 in pends]
        pens = [(emit_pre(fT), bsl_) for fT, bsl_ in flush]
        for pen, bsl_ in pens:
            emit_head2(pen, bsl_)

    nc.compile()
    return nc


_PROGRAM = None


def _get_program():
    global _PROGRAM
    if _PROGRAM is None:
        _PROGRAM = build_program()
    return _PROGRAM


def make_in_maps(inputs):
    """Host-side shard + layout prep: list of 8 per-core input maps."""
    w8, wbf, wbias = pack_weights(inputs)
    feats = [
        np.asarray(inputs["feat_text"], np.float32),
        np.asarray(inputs["feat_audio"], np.float32),
        np.asarray(inputs["feat_video"], np.float32),
    ]
    in_maps = []
    for cid in range(NCORES):
        sl = slice(cid * BL, (cid + 1) * BL)
        featT = np.stack([np.ascontiguousarray(f[sl].T) for f in feats])
        # featT: [NM, 768, 4096] -> [NM, NSTRIPES(s), 128(p), KIN(k)*512(b)]
        fs = featT.reshape(NM, KIN, 128, NSTRIPES, STRIPE)
        fs = fs.transpose(0, 3, 2, 1, 4).reshape(NM, NSTRIPES, 128, KIN * STRIPE)
        in_maps.append({
            "featS": np.ascontiguousarray(fs).astype(E4M3),
            "wmat8": w8,
            "wmatbf": wbf,
            "wbias": wbias,
        })
    return in_maps


def run_on_hw(inputs, trace=False):
    from concourse.bass_utils import run_bass_kernel_spmd

    nc = _get_program()
    in_maps = make_in_maps(inputs)
    res = run_bass_kernel_spmd(
        nc, in_maps, core_ids=list(range(NCORES)), trace=trace
    )
    out = np.concatenate([r["outT"].T for r in res.results], axis=0)
    return out, res


def kernel(**inputs):
    out, _ = run_on_hw(inputs, trace=False)
    return out
